# revision 50
# baseline (speedup 1.0000x reference)
"""Multi-head attention (B=2, S=2048, D=768, H=12, Dh=64) on 8 Trainium2 cores.

Sharding: core c handles batch b=c//4 and head-group g=c%4 (3 heads each).
Host sums the 4 partial y's per batch and applies all output biases.

v3 structural changes vs v2:
  - Head-half native layout: Q/K of h0 live on partitions 0:64, h1 on
    64:128 (Q01/K01 tiles), h2 split across Q2K2 + a single dup tile QK2d.
    One [128,W] bias-add per proj unit, only 2 dup DMAs per chunk.
  - 5 q-chunks (512,512,512,256,256): the last two are half-width so the
    epilogue drain is short.
  - PV of the last group of each chunk is carried into the next chunk
    (after its first QK trio) instead of stalling on its own exp.
  - Input DMAs round-robin across the sync/gpsimd rings; wo/bias/dups on
    the scalar ring; y-stores alternate sync/gpsimd.
  - Junk warm-up matmuls keep the PE p-state high through the prologue
    and the last-chunk norm window.
  - Norm: den + pv copies on the Scalar(ACT) engine, reciprocal on DVE
    from PSUM, O-mul on GpSimd (non-last) or DVE direct from PV PSUM
    (last); last chunk's out-proj split into 3 per-head waves that start
    as soon as that head's O is ready, accumulating in 6 live PSUM banks.
"""

import math

import numpy as np
import ml_dtypes

import concourse.bass as bass
import concourse.mybir as mybir
import concourse.tile as tile
from concourse import bacc, bass_utils
from concourse.bass import ts, ds

B, S, D = 2, 2048, 768
H, DH = 12, 64
NCORES = 8
HPC = 3
SCALE = 1.0 / math.sqrt(DH)

f32 = mybir.dt.float32
bf16 = mybir.dt.bfloat16
BF16NP = ml_dtypes.bfloat16

QC = 512
NKB = S // 128
NGRP = NKB // 2

CHUNKS = [(0, 512), (512, 512), (1024, 512), (1536, 512)]
NCH = len(CHUNKS)

# (chunk, grp, h) exp tiles computed on DVE via Schraudolph instead of ACT
OFFLOAD = ({(c, g, 1) for c in (1, 2, 3) for g in range(1, 8)}
           | {(0, g, 1) for g in (2, 3, 4, 5, 6, 7)}
           | {(c, 0, 2) for c in (1, 2, 3)})


def build_program():
    nc = bacc.Bacc("TRN2", target_bir_lowering=False, debug=False)
    qT_d = nc.dram_tensor("qT", [D, S], bf16, kind="ExternalInput").ap()
    wt_d = nc.dram_tensor("wt", [D, 576], bf16, kind="ExternalInput").ap()
    bias_d = nc.dram_tensor("biasqk", [128, 3], f32, kind="ExternalInput").ap()
    wo01_d = nc.dram_tensor("wo01", [128, D], bf16, kind="ExternalInput").ap()
    wo2_d = nc.dram_tensor("wo2", [64, D], bf16, kind="ExternalInput").ap()
    yT_d = nc.dram_tensor("yT", [D, S], bf16, kind="ExternalOutput").ap()

    with tile.TileContext(nc) as tc:
        emit(tc, nc, qT_d, wt_d, bias_d, wo01_d, wo2_d, yT_d)
    nc.compile()
    return nc


def emit(tc, nc, qT_d, wt_d, bias_d, wo01_d, wo2_d, yT_d):
    Exp = mybir.ActivationFunctionType.Exp
    yT_r = yT_d.rearrange("(o p) s -> p o s", p=128)
    qT_r = qT_d.rearrange("(o p) s -> p o s", p=128)

    import contextlib
    with contextlib.ExitStack() as octx:
        cpool = octx.enter_context(tc.tile_pool(name="cpool", bufs=1))

        scratch = cpool.tile([1, 16], f32, name="scratch")
        bias_sb = cpool.tile([128, 3], f32, name="bias_sb")
        # early activation-table load (Exp), before any real dependency
        nc.scalar.activation(scratch, scratch, Exp)
        # junk tiles for PE p-state warm-up matmuls; memset on GpSimd,
        # whose queue clears the preamble barrier earliest, so the
        # warm-ups can issue as soon as possible
        wj1 = cpool.tile([128, 128], bf16, name="wj1")
        wj2 = cpool.tile([128, QC], bf16, name="wj2")
        nc.gpsimd.memset(wj1, 0.0)
        nc.gpsimd.memset(wj2, 0.0)
        ones1 = cpool.tile([1, 64], bf16, name="ones1")
        nc.vector.memset(ones1, 1.0)
        # Schraudolph exp-approx constants (bf16 bit space):
        # i16 = x*128*log2(e) + 128*(127-c); bitcast int16 -> bf16 ~= exp(x)
        bconst = cpool.tile([128, 1], f32, name="bconst")
        nc.vector.memset(bconst, 16249.6665)

        # Q/K storage: h0 on partitions 0:64, h1 on 64:128 (native),
        # h2 split: Q2K2 rows 0:64 = Qd2, rows 64:128 = Kd2;
        # QK2d rows 0:64 = Kd2 copy, rows 64:128 = Qd2 copy.
        Q01 = cpool.tile([128, S], bf16, name="Q01")
        K01 = cpool.tile([128, S], bf16, name="K01")
        Q2K2 = cpool.tile([128, S], bf16, name="Q2K2")
        QK2d = cpool.tile([128, S], bf16, name="QK2d")
        # V in [key, col] layout; per head h cols 65h..65h+63 = V_h,
        # col 65h+64 = ones (denominator).
        V_sb = cpool.tile([128, NKB, 195], bf16, name="V_sb")
        for h in range(HPC):
            nc.vector.memset(V_sb[:, :, 65 * h + 64: 65 * h + 65], 1.0)

        ppool = octx.enter_context(tc.tile_pool(name="prep", bufs=1))

        qTc_tiles = {}
        dq_state = [0]
        DQS = None  # set below once nc engines known

        def dq():
            e = DQS[dq_state[0] % len(DQS)]
            dq_state[0] += 1
            return e

        DQS = [nc.sync]

        def get_qTc(c):
            if c not in qTc_tiles:
                qoff, W = CHUNKS[c]
                t = ppool.tile([128, 6, W], bf16, name=f"qTc{c}",
                               tag="qTc", bufs=3)
                for cc in range(6):
                    dq().dma_start(t[:, cc, :], qT_r[:, cc, ds(qoff, W)])
                qTc_tiles[c] = t
            return qTc_tiles[c]

        wt_sb = cpool.tile([128, 6, 576], bf16, name="wt_sb")
        wt_r = wt_d.rearrange("(o p) m -> p o m", p=128)
        # interleave first qT chunk with weights over 4 DMA rings so the
        # prologue matmuls are paced by compute, not a single ring
        t0 = ppool.tile([128, 6, QC], bf16, name="qTc0", tag="qTc", bufs=3)
        RING3 = [nc.sync, nc.sync]
        for cc in range(6):
            RING3[0].dma_start(wt_sb[:, cc, :], wt_r[:, cc, :])
            RING3[1].dma_start(
                t0[:, cc, :], qT_r[:, cc, ds(0, QC)])
            if cc == 1:
                nc.sync.dma_start(bias_sb, bias_d)
        qTc_tiles[0] = t0
        wo01_sb = cpool.tile([128, D], bf16, name="wo01_sb")
        wo1_sb = cpool.tile([64, D], bf16, name="wo1_sb")
        wo2_sb = cpool.tile([64, D], bf16, name="wo2_sb")
        nc.sync.dma_start(wo01_sb, wo01_d)
        nc.sync.dma_start(wo1_sb, wo01_d[64:128, :])
        nc.sync.dma_start(wo2_sb, wo2_d)

        with tc.tile_pool(name="attn", bufs=2) as apool, \
             tc.tile_pool(name="ps_s", bufs=2, space="PSUM") as psS, \
             tc.tile_pool(name="ps_pv", bufs=1, space="PSUM") as psPV, \
             tc.tile_pool(name="ps_aux", bufs=1, space="PSUM") as psA:

            def junk_mm(n=1, psum=None):
                for _ in range(n):
                    if psum is None:
                        ps = psA.tile([128, QC], f32, name="wps", tag="aux")
                    else:
                        ps = psum.tile([128, 2 * QC], f32, name="wps",
                                       tag="psc")[:, 0:QC]
                    nc.tensor.matmul(ps, lhsT=wj1, rhs=wj2)

            # ---- projection unit: one M-block x one chunk ----
            # row blocks (partition-half dsts):
            #  mi0: [Qh0 | Qh1] -> Q01,  mi1: [Qh2 | Kh2] -> Q2K2,
            #  mi2: [Kh0 | Kh1] -> K01
            PDST = {0: Q01, 1: Q2K2, 2: K01}

            def proj_mms(mi, c, ps):
                qTc = get_qTc(c)
                for cc in range(6):
                    nc.tensor.matmul(ps, lhsT=wt_sb[:, cc, ds(mi * 128, 128)],
                                     rhs=qTc[:, cc, :],
                                     start=(cc == 0), stop=(cc == 5))

            def proj_fin(mi, c, ps):
                qoff, W = CHUNKS[c]
                sl = ds(qoff, W)
                dst = PDST[mi]
                nc.vector.tensor_add(
                    dst[:, sl], ps[:, 0:W],
                    bias_sb[:, mi:mi + 1].to_broadcast((128, W)))
                if mi == 1:
                    # h2 dup halves: Kd2 -> rows 0:64, Qd2 -> rows 64:128
                    nc.sync.dma_start(QK2d[0:64, sl], dst[64:128, sl])
                    nc.sync.dma_start(QK2d[64:128, sl], dst[0:64, sl])

            def emit_proj(mi, c):
                qoff, W = CHUNKS[c]
                ps = psA.tile([128, QC], f32, name="ps", tag="aux")[:, 0:W]
                proj_mms(mi, c, ps)
                proj_fin(mi, c, ps)

            # ---- direct-V unit: two 128-key blocks per PSUM tile ----
            def emit_v2(kb0):
                ps = psA.tile([128, QC], f32, name="ps", tag="aux")
                for i, kb in enumerate((kb0, kb0 + 1)):
                    c = [c_ for c_, (qo, W_) in enumerate(CHUNKS)
                         if qo <= kb * 128 < qo + W_][0]
                    qoff, W = CHUNKS[c]
                    qTc = get_qTc(c)
                    loc = kb * 128 - qoff
                    for cc in range(6):
                        nc.tensor.matmul(
                            ps[:, ds(256 * i, 192)],
                            lhsT=qTc[:, cc, ds(loc, 128)],
                            rhs=wt_sb[:, cc, ds(384, 192)],
                            start=(cc == 0), stop=(cc == 5))
                for i, kb in enumerate((kb0, kb0 + 1)):
                    for h in range(HPC):
                        nc.vector.tensor_copy(
                            V_sb[:, kb, ds(65 * h, 64)],
                            ps[:, ds(256 * i + 64 * h, 64)])

            # ---- prologue ----
            # warm the PE while the first DMA waves land (DMA-completion
            # semaphore latency is ~1.5-2us, so bridge until cc0 is usable)
            junk_mm(5)
            # cc-interleaved: units mi2/mi0 on the two psS bufs, mi1 on aux
            ps2 = psS.tile([128, 2 * QC], f32, name="psp", tag="psc")[:, 0:QC]
            ps0 = psS.tile([128, 2 * QC], f32, name="psp", tag="psc")[:, 0:QC]
            ps1 = psA.tile([128, QC], f32, name="ps", tag="aux")
            for cc in range(6):
                for mi, ps in ((2, ps2), (0, ps0), (1, ps1)):
                    nc.tensor.matmul(ps,
                                     lhsT=wt_sb[:, cc, ds(mi * 128, 128)],
                                     rhs=t0[:, cc, :],
                                     start=(cc == 0), stop=(cc == 5))
                if cc < 5:
                    # keep-warm fills while the next cc DMA pair lands
                    # (PV banks are free until the chunk loop starts)
                    for i in range(2):
                        jp = psPV.tile([128, QC], f32, name="wpv",
                                       tag=f"pv{(2 * cc + i) % 3}")
                        nc.tensor.matmul(jp, lhsT=wj1, rhs=wj2,
                                         skip_group_check=True)
            for mi, ps in ((2, ps2), (0, ps0), (1, ps1)):
                proj_fin(mi, 0, ps)

            # deferred unit schedule: slot = 8*ci + grp
            deferred = [
                [("V", 0), ("V", 2)],                           # c0 g0
                [("P", 2, 1), ("P", 1, 1), ("Q", 2)],           # c0 g1
                [("V", 4)],                                     # c0 g2
                [("P", 2, 2), ("P", 1, 2), ("P", 0, 1)],        # c0 g3
                [("V", 6), ("V", 8), ("Q", 3)],                 # c0 g4
                [("P", 2, 3), ("P", 1, 3)],                     # c0 g5
                [("V", 10), ("V", 12)],                         # c0 g6
                [("V", 14)],                                    # c0 g7
                [], [], [], [],                                 # c1 g0-3
                [("P", 0, 2)],                                  # c1 g4
                [], [], [], [],                                 # c1 g5-7, c2 g0
                [("P", 0, 3)],                                  # c2 g1
            ]

            def pump_deferred(slot):
                if slot < len(deferred):
                    for unit in deferred[slot]:
                        if unit[0] == "V":
                            emit_v2(unit[1])
                        elif unit[0] == "Q":
                            get_qTc(unit[1])
                        else:
                            emit_proj(unit[1], unit[2])

            # ---- norm + output projection for one q-chunk ----
            def norm_steps(pv, qoff, W, last):
                qsl = ds(qoff, W)
                denb = apool.tile([1, HPC, QC], bf16, name="denb", tag="denb")
                recs = apool.tile([64, HPC, QC], f32, name="recs", tag="recs")
                Ost = apool.tile([128, QC], bf16, name="Ost", tag="Ost")
                Oh1 = apool.tile([64, QC], bf16, name="Oh1", tag="Oh1")
                Oh2 = apool.tile([64, QC], bf16, name="Oh2", tag="Oh2")
                if not last:
                    pvc = apool.tile([64, HPC, QC], f32, name="pvc",
                                     tag="pvc")

                def brc(h):
                    bcD = psA.tile([128, QC], f32, name="bcD", tag="aux")
                    nc.tensor.matmul(bcD[0:64, 0:W], lhsT=ones1,
                                     rhs=denb[:, h, 0:W])
                    nc.vector.reciprocal_approx_fast(recs[:, h, 0:W],
                                                     bcD[0:64, 0:W])

                Odst = (Ost[0:64], Oh1, Oh2)
                steps = []
                if last:
                    # 3-wave yproj: per-head partial accumulation into six
                    # live PSUM banks; waves start as each head's O lands.
                    yps = {}
                    for jb in range(3):
                        yps[jb] = psPV.tile([128, QC], f32, name="ypv",
                                            tag=f"pv{jb}")
                    y34 = psS.tile([128, 2 * QC], f32, name="ypsc",
                                   tag="psc")
                    yps[3], yps[4] = y34[:, 0:QC], y34[:, QC:2 * QC]
                    yps[5] = psS.tile([128, 2 * QC], f32, name="ypsc",
                                      tag="psc")[:, 0:QC]

                    def junk_y(jb):
                        # p-state keep-warm: junk matmul into a yps bank
                        # before its real accumulation resets it
                        nc.tensor.matmul(yps[jb], lhsT=wj1, rhs=wj2,
                                         skip_group_check=True)

                for h in range(HPC):
                    steps.append(lambda h=h: nc.scalar.copy(
                        denb[:, h, 0:W], pv[h][64:65, 0:W]))
                    if last:
                        steps.append(lambda h=h: junk_y(2 * h))
                        steps.append(lambda h=h: brc(h))
                        steps.append(lambda h=h: junk_y(2 * h + 1))
                        steps.append(lambda h=h: nc.vector.tensor_mul(
                            Odst[h][:, 0:W], pv[h][0:64, 0:W],
                            recs[:, h, 0:W]))
                    else:
                        steps.append(lambda h=h: nc.vector.tensor_copy(
                            pvc[:, h, 0:W], pv[h][0:64, 0:W]))
                        steps.append(lambda h=h: brc(h))
                        steps.append(lambda h=h: nc.gpsimd.tensor_mul(
                            Odst[h][:, 0:W], pvc[:, h, 0:W],
                            recs[:, h, 0:W]))

                if not last:
                    def oh1_move():
                        nc.sync.dma_start(Ost[64:128, 0:W], Oh1[:, 0:W])
                    steps.insert(8, oh1_move)

                if last:
                    # per-head 64-contraction waves; wo1 lives in its own
                    # base-0 tile so no partition-shift DMA is needed
                    WSRC = ((wo01_sb[0:64], Ost[0:64]), (wo1_sb, Oh1),
                            (wo2_sb, Oh2))

                    def wave(part, jb):
                        w, o = WSRC[part]
                        nc.tensor.matmul(
                            yps[jb][:, 0:W], lhsT=w[:, ts(jb, 128)],
                            rhs=o[:, 0:W],
                            start=(part == 0), stop=(part == 2),
                            skip_group_check=True)

                    def fin_jb(jb):
                        ysb = apool.tile([128, QC], bf16, name="ysbL",
                                         tag="ysbL", bufs=6)
                        nc.vector.tensor_copy(ysb[:, 0:W],
                                              yps[jb][:, 0:W])
                        nc.sync.dma_start(yT_r[:, jb, qsl], ysb[:, 0:W])

                    steps += [lambda jb=jb: wave(0, jb) for jb in range(6)]
                    steps += [lambda jb=jb: wave(1, jb) for jb in range(6)]
                    steps += [lambda jb=jb: wave(2, jb) for jb in range(6)]
                    steps += [lambda jb=jb: fin_jb(jb) for jb in range(6)]
                else:
                    def proj_jb(jb):
                        yps = psA.tile([128, QC], f32, name="yps", tag="aux")
                        nc.tensor.matmul(yps[:, 0:W],
                                         lhsT=wo01_sb[:, ts(jb, 128)],
                                         rhs=Ost[:, 0:W],
                                         start=True, stop=False)
                        nc.tensor.matmul(yps[:, 0:W],
                                         lhsT=wo2_sb[:, ts(jb, 128)],
                                         rhs=Oh2[:, 0:W],
                                         start=False, stop=True)
                        ysb = apool.tile([128, QC], bf16, name="ysb",
                                         tag="ysb", bufs=3)
                        nc.vector.tensor_copy(ysb[:, 0:W], yps[:, 0:W])
                        eng = nc.sync
                        eng.dma_start(yT_r[:, jb, qsl], ysb[:, 0:W])

                    steps += [lambda jb=jb: proj_jb(jb) for jb in range(6)]
                return steps

            pending = []
            carry_pv = None
            for ci, (qoff, W) in enumerate(CHUNKS):
                qsl = ds(qoff, W)
                slot0 = 8 * ci
                last = ci == NCH - 1
                attn = [apool.tile([128, NKB * QC], bf16,
                                   name=f"attn{h}", tag=f"attn{h}")
                        for h in range(HPC)]
                pv = [psPV.tile([128, QC], f32, name=f"pv{h}", tag=f"pv{h}")
                      for h in range(HPC)]

                def emit_pv(g, pv=pv, attn=attn, W=W):
                    for h in range(HPC):
                        for kb in (2 * g, 2 * g + 1):
                            nc.tensor.matmul(
                                pv[h][0:65, 0:W],
                                lhsT=V_sb[:, kb, ds(65 * h, 65)],
                                rhs=attn[h][:, kb * W:(kb + 1) * W],
                                start=(kb == 0), stop=(kb == NKB - 1),
                                skip_group_check=True)

                # QK operand map: per head, (lhsT tile+rows, rhs tile+rows)
                # h0 rows 0:64, h1 rows 64:128 (native); h2 alternates via
                # the dup tile so consecutive matmuls ping-pong PE row tiles
                def qk(h, kb, even):
                    if h == 0:
                        return K01[0:64, ts(kb, 128)], Q01[0:64, qsl]
                    if h == 1:
                        return K01[64:128, ts(kb, 128)], Q01[64:128, qsl]
                    if even:
                        return QK2d[0:64, ts(kb, 128)], Q2K2[0:64, qsl]
                    return Q2K2[64:128, ts(kb, 128)], QK2d[64:128, qsl]

                for grp in range(NGRP):
                    if grp == 0 and carry_pv is not None:
                        carry_pv()
                        carry_pv = None
                    kb0, kb1 = 2 * grp, 2 * grp + 1
                    for h in range(HPC):
                        psc = psS.tile([128, 2 * QC], f32, name="psc",
                                       tag="psc")
                        l0, r0 = qk(h, kb0, True)
                        l1, r1 = qk(h, kb1, False)
                        nc.tensor.matmul(psc[:, 0:W], lhsT=l0, rhs=r0)
                        nc.tensor.matmul(psc[:, QC:QC + W], lhsT=l1, rhs=r1)
                        ob = attn[h][:, grp * 2 * W:(grp + 1) * 2 * W]
                        if W == QC:
                            if (ci, grp, h) in OFFLOAD:
                                nc.vector.scalar_tensor_tensor(
                                    ob.bitcast(mybir.dt.int16),
                                    psc, 184.6650292,
                                    bconst.to_broadcast((128, 2 * W)),
                                    mybir.AluOpType.mult,
                                    mybir.AluOpType.add)
                            else:
                                nc.scalar.activation(ob, psc, Exp)
                        elif (ci, grp, h) in OFFLOAD:
                            for i2 in range(2):
                                nc.vector.scalar_tensor_tensor(
                                    ob[:, i2 * W:(i2 + 1) * W]
                                    .bitcast(mybir.dt.int16),
                                    psc[:, i2 * QC:i2 * QC + W],
                                    184.6650292,
                                    bconst.to_broadcast((128, W)),
                                    mybir.AluOpType.mult,
                                    mybir.AluOpType.add)
                        else:
                            for i2 in range(2):
                                nc.scalar.activation(
                                    ob[:, i2 * W:(i2 + 1) * W],
                                    psc[:, i2 * QC:i2 * QC + W], Exp)
                    pump_deferred(slot0 + grp)
                    if last:
                        emit_pv(grp)
                    elif grp > 0:
                        emit_pv(grp - 1)
                    for _ in range(3):
                        if pending:
                            pending.pop(0)()
                if not last:
                    carry_pv = lambda g=NGRP - 1, f=emit_pv: f(g)
                while pending:
                    pending.pop(0)()
                pending = norm_steps(pv, qoff, W, last)
            while pending:
                pending.pop(0)()


# ---------------------------------------------------------------------------
# host side
# ---------------------------------------------------------------------------

def make_core_inputs(q, W_qkv, b_qkv, W_out, b_out):
    q = np.asarray(q, np.float32)
    W_qkv = np.asarray(W_qkv, np.float32)
    b_qkv = np.asarray(b_qkv, np.float32)
    W_out = np.asarray(W_out, np.float32)

    Wq, Wk, Wv = W_qkv[0:D], W_qkv[D:2 * D], W_qkv[2 * D:3 * D]
    bq, bk = b_qkv[0:D], b_qkv[D:2 * D]

    def hrows(W, h):
        return W[h * DH:(h + 1) * DH]

    def hbias(bvec, h):
        return bvec[h * DH:(h + 1) * DH]

    in_maps = []
    for c in range(NCORES):
        b = c // 4
        g = c % 4
        h0, h1, h2 = 3 * g, 3 * g + 1, 3 * g + 2

        qT = np.ascontiguousarray(q[b].T).astype(BF16NP)

        wt = np.concatenate([
            hrows(Wq, h0) * SCALE, hrows(Wq, h1) * SCALE,
            hrows(Wq, h2) * SCALE, hrows(Wk, h2),
            hrows(Wk, h0), hrows(Wk, h1),
            hrows(Wv, h0), hrows(Wv, h1), hrows(Wv, h2),
        ], axis=0)
        wt = np.ascontiguousarray(wt.T).astype(BF16NP)

        biasqk = np.stack([
            np.concatenate([hbias(bq, h0), hbias(bq, h1)]) * SCALE,
            np.concatenate([hbias(bq, h2) * SCALE, hbias(bk, h2)]),
            np.concatenate([hbias(bk, h0), hbias(bk, h1)]),
        ], axis=1).astype(np.float32)

        wo01 = np.concatenate([
            W_out[:, h0 * DH:(h0 + 1) * DH].T,
            W_out[:, h1 * DH:(h1 + 1) * DH].T,
        ], axis=0)
        wo01 = np.ascontiguousarray(wo01).astype(BF16NP)
        wo2 = np.ascontiguousarray(
            W_out[:, h2 * DH:(h2 + 1) * DH].T).astype(BF16NP)

        in_maps.append({
            "qT": qT, "wt": wt, "biasqk": biasqk,
            "wo01": wo01, "wo2": wo2,
        })
    return in_maps


_NC = None


def _get_nc():
    global _NC
    if _NC is None:
        _NC = build_program()
    return _NC


def kernel(q, k, v, W_qkv, b_qkv, W_out, b_out, _trace=False):
    nc = _get_nc()
    in_maps = make_core_inputs(q, W_qkv, b_qkv, W_out, b_out)
    res = bass_utils.run_bass_kernel_spmd(
        nc, in_maps, core_ids=list(range(NCORES)), trace=_trace)
    kernel.last_result = res
    W_out = np.asarray(W_out, np.float32)
    bv = np.asarray(b_qkv, np.float32)[2 * D:3 * D]
    bias = np.asarray(b_out, np.float32) + W_out @ bv
    y = np.empty((B, S, D), np.float32)
    for b in range(B):
        acc = res.results[4 * b]["yT"].astype(np.float32)
        for g in range(1, 4):
            acc = acc + res.results[4 * b + g]["yT"]
        y[b] = acc.T + bias
    return y


# revision 52
# speedup vs baseline: 1.0174x; 1.0174x over previous
"""Multi-head attention (B=2, S=2048, D=768, H=12, Dh=64) on 8 Trainium2 cores.

Sharding: core c handles batch b=c//4 and head-group g=c%4 (3 heads each).
Host sums the 4 partial y's per batch and applies all output biases.

v3 structural changes vs v2:
  - Head-half native layout: Q/K of h0 live on partitions 0:64, h1 on
    64:128 (Q01/K01 tiles), h2 split across Q2K2 + a single dup tile QK2d.
    One [128,W] bias-add per proj unit, only 2 dup DMAs per chunk.
  - 5 q-chunks (512,512,512,256,256): the last two are half-width so the
    epilogue drain is short.
  - PV of the last group of each chunk is carried into the next chunk
    (after its first QK trio) instead of stalling on its own exp.
  - Input DMAs round-robin across the sync/gpsimd rings; wo/bias/dups on
    the scalar ring; y-stores alternate sync/gpsimd.
  - Junk warm-up matmuls keep the PE p-state high through the prologue
    and the last-chunk norm window.
  - Norm: den + pv copies on the Scalar(ACT) engine, reciprocal on DVE
    from PSUM, O-mul on GpSimd (non-last) or DVE direct from PV PSUM
    (last); last chunk's out-proj split into 3 per-head waves that start
    as soon as that head's O is ready, accumulating in 6 live PSUM banks.
"""

import math

import numpy as np
import ml_dtypes

import concourse.bass as bass
import concourse.mybir as mybir
import concourse.tile as tile
from concourse import bacc, bass_utils
from concourse.bass import ts, ds

B, S, D = 2, 2048, 768
H, DH = 12, 64
NCORES = 8
HPC = 3
SCALE = 1.0 / math.sqrt(DH)

f32 = mybir.dt.float32
bf16 = mybir.dt.bfloat16
BF16NP = ml_dtypes.bfloat16

QC = 512
NKB = S // 128
NGRP = NKB // 2

CHUNKS = [(0, 512), (512, 512), (1024, 512), (1536, 512)]
NCH = len(CHUNKS)

# (chunk, grp, h) exp tiles computed on DVE via Schraudolph instead of ACT
OFFLOAD = ({(c, g, 1) for c in (1, 2, 3) for g in range(1, 8)}
           | {(0, g, 1) for g in (2, 3, 4, 5, 6, 7)}
           | {(c, 0, 2) for c in (1, 2, 3)})


def build_program():
    nc = bacc.Bacc("TRN2", target_bir_lowering=False, debug=False)
    qT_d = nc.dram_tensor("qT", [D, S], bf16, kind="ExternalInput").ap()
    wt_d = nc.dram_tensor("wt", [D, 576], bf16, kind="ExternalInput").ap()
    bias_d = nc.dram_tensor("biasqk", [128, 3], f32, kind="ExternalInput").ap()
    wo01_d = nc.dram_tensor("wo01", [128, D], bf16, kind="ExternalInput").ap()
    wo2_d = nc.dram_tensor("wo2", [64, D], bf16, kind="ExternalInput").ap()
    yT_d = nc.dram_tensor("yT", [D, S], bf16, kind="ExternalOutput").ap()

    with tile.TileContext(nc) as tc:
        emit(tc, nc, qT_d, wt_d, bias_d, wo01_d, wo2_d, yT_d)
    nc.compile()
    return nc


def emit(tc, nc, qT_d, wt_d, bias_d, wo01_d, wo2_d, yT_d):
    Exp = mybir.ActivationFunctionType.Exp
    yT_r = yT_d.rearrange("(o p) s -> p o s", p=128)
    qT_r = qT_d.rearrange("(o p) s -> p o s", p=128)

    import contextlib
    with contextlib.ExitStack() as octx:
        cpool = octx.enter_context(tc.tile_pool(name="cpool", bufs=1))

        scratch = cpool.tile([1, 16], f32, name="scratch")
        bias_sb = cpool.tile([128, 3], f32, name="bias_sb")
        # early activation-table load (Exp), before any real dependency
        nc.scalar.activation(scratch, scratch, Exp)
        # junk tiles for PE p-state warm-up matmuls; memset on GpSimd,
        # whose queue clears the preamble barrier earliest, so the
        # warm-ups can issue as soon as possible
        wj1 = cpool.tile([128, 128], bf16, name="wj1")
        wj2 = cpool.tile([128, QC], bf16, name="wj2")
        nc.gpsimd.memset(wj1, 0.0)
        nc.gpsimd.memset(wj2, 0.0)
        ones1 = cpool.tile([1, 64], bf16, name="ones1")
        nc.vector.memset(ones1, 1.0)
        # Schraudolph exp-approx constants (bf16 bit space):
        # i16 = x*128*log2(e) + 128*(127-c); bitcast int16 -> bf16 ~= exp(x)
        bconst = cpool.tile([128, 1], f32, name="bconst")
        nc.vector.memset(bconst, 16249.6665)

        # Q/K storage: h0 on partitions 0:64, h1 on 64:128 (native),
        # h2 split: Q2K2 rows 0:64 = Qd2, rows 64:128 = Kd2;
        # QK2d rows 0:64 = Kd2 copy, rows 64:128 = Qd2 copy.
        Q01 = cpool.tile([128, S], bf16, name="Q01")
        K01 = cpool.tile([128, S], bf16, name="K01")
        Q2K2 = cpool.tile([128, S], bf16, name="Q2K2")
        QK2d = cpool.tile([128, S], bf16, name="QK2d")
        # V in [key, col] layout; per head h cols 65h..65h+63 = V_h,
        # col 65h+64 = ones (denominator).
        V_sb = cpool.tile([128, NKB, 195], bf16, name="V_sb")
        for h in range(HPC):
            nc.vector.memset(V_sb[:, :, 65 * h + 64: 65 * h + 65], 1.0)

        ppool = octx.enter_context(tc.tile_pool(name="prep", bufs=1))

        qTc_tiles = {}
        dq_state = [0]
        DQS = None  # set below once nc engines known

        def dq():
            e = DQS[dq_state[0] % len(DQS)]
            dq_state[0] += 1
            return e

        DQS = [nc.sync]

        def get_qTc(c):
            if c not in qTc_tiles:
                qoff, W = CHUNKS[c]
                t = ppool.tile([128, 6, W], bf16, name=f"qTc{c}",
                               tag="qTc", bufs=3)
                for cc in range(6):
                    dq().dma_start(t[:, cc, :], qT_r[:, cc, ds(qoff, W)])
                qTc_tiles[c] = t
            return qTc_tiles[c]

        wt_sb = cpool.tile([128, 6, 576], bf16, name="wt_sb")
        wt_r = wt_d.rearrange("(o p) m -> p o m", p=128)
        # interleave first qT chunk with weights over 4 DMA rings so the
        # prologue matmuls are paced by compute, not a single ring
        t0 = ppool.tile([128, 6, QC], bf16, name="qTc0", tag="qTc", bufs=3)
        RING3 = [nc.sync, nc.sync]
        for cc in range(6):
            RING3[0].dma_start(wt_sb[:, cc, :], wt_r[:, cc, :])
            RING3[1].dma_start(
                t0[:, cc, :], qT_r[:, cc, ds(0, QC)])
            if cc == 1:
                nc.sync.dma_start(bias_sb, bias_d)
        qTc_tiles[0] = t0
        wo01_sb = cpool.tile([128, D], bf16, name="wo01_sb")
        wo1_sb = cpool.tile([64, D], bf16, name="wo1_sb")
        wo2_sb = cpool.tile([64, D], bf16, name="wo2_sb")
        nc.sync.dma_start(wo01_sb, wo01_d)
        nc.sync.dma_start(wo1_sb, wo01_d[64:128, :])
        nc.sync.dma_start(wo2_sb, wo2_d)

        with tc.tile_pool(name="attn", bufs=2) as apool, \
             tc.tile_pool(name="ps_s", bufs=2, space="PSUM") as psS, \
             tc.tile_pool(name="ps_pv", bufs=1, space="PSUM") as psPV, \
             tc.tile_pool(name="ps_aux", bufs=1, space="PSUM") as psA:

            def junk_mm(n=1, psum=None):
                for _ in range(n):
                    if psum is None:
                        ps = psA.tile([128, QC], f32, name="wps", tag="aux")
                    else:
                        ps = psum.tile([128, 2 * QC], f32, name="wps",
                                       tag="psc")[:, 0:QC]
                    nc.tensor.matmul(ps, lhsT=wj1, rhs=wj2)

            # ---- projection unit: one M-block x one chunk ----
            # row blocks (partition-half dsts):
            #  mi0: [Qh0 | Qh1] -> Q01,  mi1: [Qh2 | Kh2] -> Q2K2,
            #  mi2: [Kh0 | Kh1] -> K01
            PDST = {0: Q01, 1: Q2K2, 2: K01}

            def proj_mms(mi, c, ps):
                qTc = get_qTc(c)
                for cc in range(6):
                    nc.tensor.matmul(ps, lhsT=wt_sb[:, cc, ds(mi * 128, 128)],
                                     rhs=qTc[:, cc, :],
                                     start=(cc == 0), stop=(cc == 5))

            def proj_fin(mi, c, ps):
                qoff, W = CHUNKS[c]
                sl = ds(qoff, W)
                dst = PDST[mi]
                nc.vector.tensor_add(
                    dst[:, sl], ps[:, 0:W],
                    bias_sb[:, mi:mi + 1].to_broadcast((128, W)))
                if mi == 1:
                    # h2 dup halves: Kd2 -> rows 0:64, Qd2 -> rows 64:128
                    nc.sync.dma_start(QK2d[0:64, sl], dst[64:128, sl])
                    nc.sync.dma_start(QK2d[64:128, sl], dst[0:64, sl])

            def emit_proj(mi, c):
                qoff, W = CHUNKS[c]
                ps = psA.tile([128, QC], f32, name="ps", tag="aux")[:, 0:W]
                proj_mms(mi, c, ps)
                proj_fin(mi, c, ps)

            # ---- direct-V unit: two 128-key blocks per PSUM tile ----
            def emit_v2(kb0):
                ps = psA.tile([128, QC], f32, name="ps", tag="aux")
                for i, kb in enumerate((kb0, kb0 + 1)):
                    c = [c_ for c_, (qo, W_) in enumerate(CHUNKS)
                         if qo <= kb * 128 < qo + W_][0]
                    qoff, W = CHUNKS[c]
                    qTc = get_qTc(c)
                    loc = kb * 128 - qoff
                    for cc in range(6):
                        nc.tensor.matmul(
                            ps[:, ds(256 * i, 192)],
                            lhsT=qTc[:, cc, ds(loc, 128)],
                            rhs=wt_sb[:, cc, ds(384, 192)],
                            start=(cc == 0), stop=(cc == 5))
                for i, kb in enumerate((kb0, kb0 + 1)):
                    for h in range(HPC):
                        nc.vector.tensor_copy(
                            V_sb[:, kb, ds(65 * h, 64)],
                            ps[:, ds(256 * i + 64 * h, 64)])

            # ---- prologue ----
            # warm the PE while the first DMA waves land (DMA-completion
            # semaphore latency is ~1.5-2us, so bridge until cc0 is usable)
            junk_mm(5)
            # cc-interleaved: units mi2/mi0 on the two psS bufs, mi1 on aux
            ps2 = psS.tile([128, 2 * QC], f32, name="psp", tag="psc")[:, 0:QC]
            ps0 = psS.tile([128, 2 * QC], f32, name="psp", tag="psc")[:, 0:QC]
            ps1 = psA.tile([128, QC], f32, name="ps", tag="aux")
            for cc in range(6):
                for mi, ps in ((2, ps2), (0, ps0), (1, ps1)):
                    nc.tensor.matmul(ps,
                                     lhsT=wt_sb[:, cc, ds(mi * 128, 128)],
                                     rhs=t0[:, cc, :],
                                     start=(cc == 0), stop=(cc == 5))
                if cc < 5:
                    # keep-warm fills while the next cc DMA pair lands
                    # (PV banks are free until the chunk loop starts)
                    for i in range(2):
                        jp = psPV.tile([128, QC], f32, name="wpv",
                                       tag=f"pv{(2 * cc + i) % 3}")
                        nc.tensor.matmul(jp, lhsT=wj1, rhs=wj2,
                                         skip_group_check=True)
            for mi, ps in ((2, ps2), (0, ps0), (1, ps1)):
                proj_fin(mi, 0, ps)

            # deferred unit schedule: slot = 8*ci + grp
            deferred = [
                [("V", 0), ("V", 2)],                           # c0 g0
                [("P", 2, 1), ("P", 1, 1), ("Q", 2)],           # c0 g1
                [("V", 4)],                                     # c0 g2
                [("P", 2, 2), ("P", 1, 2), ("P", 0, 1)],        # c0 g3
                [("V", 6), ("V", 8), ("Q", 3)],                 # c0 g4
                [("P", 2, 3), ("P", 1, 3)],                     # c0 g5
                [("V", 10), ("V", 12)],                         # c0 g6
                [("V", 14)],                                    # c0 g7
                [], [], [], [],                                 # c1 g0-3
                [("P", 0, 2)],                                  # c1 g4
                [], [], [], [],                                 # c1 g5-7, c2 g0
                [("P", 0, 3)],                                  # c2 g1
            ]

            def pump_deferred(slot):
                if slot < len(deferred):
                    for unit in deferred[slot]:
                        if unit[0] == "V":
                            emit_v2(unit[1])
                        elif unit[0] == "Q":
                            get_qTc(unit[1])
                        else:
                            emit_proj(unit[1], unit[2])

            # ---- norm + output projection for one q-chunk ----
            def norm_steps(pv, qoff, W, last):
                qsl = ds(qoff, W)
                denb = apool.tile([1, HPC, QC], bf16, name="denb", tag="denb")
                recs = apool.tile([64, HPC, QC], f32, name="recs", tag="recs")
                Ost = apool.tile([128, QC], bf16, name="Ost", tag="Ost")
                Oh1 = apool.tile([64, QC], bf16, name="Oh1", tag="Oh1")
                Oh2 = apool.tile([64, QC], bf16, name="Oh2", tag="Oh2")
                if not last:
                    pvc = apool.tile([64, HPC, QC], f32, name="pvc",
                                     tag="pvc")

                def brc(h):
                    bcD = psA.tile([128, QC], f32, name="bcD", tag="aux")
                    nc.tensor.matmul(bcD[0:64, 0:W], lhsT=ones1,
                                     rhs=denb[:, h, 0:W])
                    nc.vector.reciprocal_approx_fast(recs[:, h, 0:W],
                                                     bcD[0:64, 0:W])

                Odst = (Ost[0:64], Oh1, Oh2)
                steps = []
                if last:
                    # 3-wave yproj: per-head partial accumulation into six
                    # live PSUM banks; waves start as each head's O lands.
                    yps = {}
                    for jb in range(3):
                        yps[jb] = psPV.tile([128, QC], f32, name="ypv",
                                            tag=f"pv{jb}")
                    y34 = psS.tile([128, 2 * QC], f32, name="ypsc",
                                   tag="psc")
                    yps[3], yps[4] = y34[:, 0:QC], y34[:, QC:2 * QC]
                    yps[5] = psS.tile([128, 2 * QC], f32, name="ypsc",
                                      tag="psc")[:, 0:QC]

                    def junk_y(jb):
                        # p-state keep-warm: junk matmul into a yps bank
                        # before its real accumulation resets it
                        nc.tensor.matmul(yps[jb], lhsT=wj1, rhs=wj2,
                                         skip_group_check=True)

                for h in range(HPC):
                    steps.append(lambda h=h: nc.scalar.copy(
                        denb[:, h, 0:W], pv[h][64:65, 0:W]))
                    if last:
                        steps.append(lambda h=h: junk_y(2 * h))
                        steps.append(lambda h=h: brc(h))
                        steps.append(lambda h=h: junk_y(2 * h + 1))
                        steps.append(lambda h=h: nc.vector.tensor_mul(
                            Odst[h][:, 0:W], pv[h][0:64, 0:W],
                            recs[:, h, 0:W]))
                    else:
                        steps.append(lambda h=h: nc.vector.tensor_copy(
                            pvc[:, h, 0:W], pv[h][0:64, 0:W]))
                        steps.append(lambda h=h: brc(h))
                        steps.append(lambda h=h: nc.gpsimd.tensor_mul(
                            Odst[h][:, 0:W], pvc[:, h, 0:W],
                            recs[:, h, 0:W]))

                if not last:
                    def oh1_move():
                        nc.sync.dma_start(Ost[64:128, 0:W], Oh1[:, 0:W])
                    steps.insert(8, oh1_move)

                if last:
                    # per-head 64-contraction waves; wo1 lives in its own
                    # base-0 tile so no partition-shift DMA is needed
                    WSRC = ((wo01_sb[0:64], Ost[0:64]), (wo1_sb, Oh1),
                            (wo2_sb, Oh2))

                    def wave(part, jb):
                        w, o = WSRC[part]
                        nc.tensor.matmul(
                            yps[jb][:, 0:W], lhsT=w[:, ts(jb, 128)],
                            rhs=o[:, 0:W],
                            start=(part == 0), stop=(part == 2),
                            skip_group_check=True)

                    def fin_jb(jb):
                        ysb = apool.tile([128, QC], bf16, name="ysbL",
                                         tag="ysbL", bufs=6)
                        nc.vector.tensor_copy(ysb[:, 0:W],
                                              yps[jb][:, 0:W])
                        nc.sync.dma_start(yT_r[:, jb, qsl], ysb[:, 0:W])

                    steps += [lambda jb=jb: wave(0, jb) for jb in range(6)]
                    steps += [lambda jb=jb: wave(1, jb) for jb in range(6)]
                    steps += [lambda jb=jb: wave(2, jb) for jb in range(6)]
                    steps += [lambda jb=jb: fin_jb(jb) for jb in range(6)]
                else:
                    def proj_jb(jb):
                        yps = psA.tile([128, QC], f32, name="yps", tag="aux")
                        nc.tensor.matmul(yps[:, 0:W],
                                         lhsT=wo01_sb[:, ts(jb, 128)],
                                         rhs=Ost[:, 0:W],
                                         start=True, stop=False)
                        nc.tensor.matmul(yps[:, 0:W],
                                         lhsT=wo2_sb[:, ts(jb, 128)],
                                         rhs=Oh2[:, 0:W],
                                         start=False, stop=True)
                        ysb = apool.tile([128, QC], bf16, name="ysb",
                                         tag="ysb", bufs=3)
                        nc.vector.tensor_copy(ysb[:, 0:W], yps[:, 0:W])
                        eng = nc.sync
                        eng.dma_start(yT_r[:, jb, qsl], ysb[:, 0:W])

                    steps += [lambda jb=jb: proj_jb(jb) for jb in range(6)]
                return steps

            pending = []
            carry_pv = None
            for ci, (qoff, W) in enumerate(CHUNKS):
                qsl = ds(qoff, W)
                slot0 = 8 * ci
                last = ci == NCH - 1
                attn = [apool.tile([128, NKB * QC], bf16,
                                   name=f"attn{h}", tag=f"attn{h}")
                        for h in range(HPC)]
                pv = [psPV.tile([128, QC], f32, name=f"pv{h}", tag=f"pv{h}")
                      for h in range(HPC)]

                def emit_pv(g, pv=pv, attn=attn, W=W):
                    for h in range(HPC):
                        for kb in (2 * g, 2 * g + 1):
                            nc.tensor.matmul(
                                pv[h][0:65, 0:W],
                                lhsT=V_sb[:, kb, ds(65 * h, 65)],
                                rhs=attn[h][:, kb * W:(kb + 1) * W],
                                start=(kb == 0), stop=(kb == NKB - 1),
                                skip_group_check=True)

                # QK operand map: per head, (lhsT tile+rows, rhs tile+rows)
                # h0 rows 0:64, h1 rows 64:128 (native); h2 alternates via
                # the dup tile so consecutive matmuls ping-pong PE row tiles
                def qk(h, kb, even):
                    if h == 0:
                        return K01[0:64, ts(kb, 128)], Q01[0:64, qsl]
                    if h == 1:
                        return K01[64:128, ts(kb, 128)], Q01[64:128, qsl]
                    if even:
                        return QK2d[0:64, ts(kb, 128)], Q2K2[0:64, qsl]
                    return Q2K2[64:128, ts(kb, 128)], QK2d[64:128, qsl]

                for grp in range(NGRP):
                    if grp == 0 and carry_pv is not None:
                        carry_pv()
                        carry_pv = None
                    kb0, kb1 = 2 * grp, 2 * grp + 1
                    for h in range(HPC):
                        psc = psS.tile([128, 2 * QC], f32, name="psc",
                                       tag="psc")
                        l0, r0 = qk(h, kb0, True)
                        l1, r1 = qk(h, kb1, False)
                        nc.tensor.matmul(psc[:, 0:W], lhsT=l0, rhs=r0)
                        nc.tensor.matmul(psc[:, QC:QC + W], lhsT=l1, rhs=r1)
                        ob = attn[h][:, grp * 2 * W:(grp + 1) * 2 * W]
                        if W == QC:
                            if (ci, grp, h) in OFFLOAD:
                                nc.vector.scalar_tensor_tensor(
                                    ob.bitcast(mybir.dt.int16),
                                    psc, 184.6650292,
                                    bconst.to_broadcast((128, 2 * W)),
                                    mybir.AluOpType.mult,
                                    mybir.AluOpType.add)
                            else:
                                nc.scalar.activation(ob, psc, Exp)
                        elif (ci, grp, h) in OFFLOAD:
                            for i2 in range(2):
                                nc.vector.scalar_tensor_tensor(
                                    ob[:, i2 * W:(i2 + 1) * W]
                                    .bitcast(mybir.dt.int16),
                                    psc[:, i2 * QC:i2 * QC + W],
                                    184.6650292,
                                    bconst.to_broadcast((128, W)),
                                    mybir.AluOpType.mult,
                                    mybir.AluOpType.add)
                        else:
                            for i2 in range(2):
                                nc.scalar.activation(
                                    ob[:, i2 * W:(i2 + 1) * W],
                                    psc[:, i2 * QC:i2 * QC + W], Exp)
                    pump_deferred(slot0 + grp)
                    if last:
                        emit_pv(grp)
                    elif grp > 0:
                        emit_pv(grp - 1)
                    for _ in range(3):
                        if pending:
                            pending.pop(0)()
                if not last:
                    carry_pv = lambda g=NGRP - 1, f=emit_pv: f(g)
                while pending:
                    pending.pop(0)()
                pending = norm_steps(pv, qoff, W, last)
            while pending:
                pending.pop(0)()


# ---------------------------------------------------------------------------
# host side
# ---------------------------------------------------------------------------

def make_core_inputs(q, W_qkv, b_qkv, W_out, b_out):
    q = np.asarray(q, np.float32)
    W_qkv = np.asarray(W_qkv, np.float32)
    b_qkv = np.asarray(b_qkv, np.float32)
    W_out = np.asarray(W_out, np.float32)

    Wq, Wk, Wv = W_qkv[0:D], W_qkv[D:2 * D], W_qkv[2 * D:3 * D]
    bq, bk = b_qkv[0:D], b_qkv[D:2 * D]

    def hrows(W, h):
        return W[h * DH:(h + 1) * DH]

    def hbias(bvec, h):
        return bvec[h * DH:(h + 1) * DH]

    in_maps = []
    for c in range(NCORES):
        b = c // 4
        g = c % 4
        h0, h1, h2 = 3 * g, 3 * g + 1, 3 * g + 2

        qT = np.ascontiguousarray(q[b].T).astype(BF16NP)

        wt = np.concatenate([
            hrows(Wq, h0) * SCALE, hrows(Wq, h1) * SCALE,
            hrows(Wq, h2) * SCALE, hrows(Wk, h2),
            hrows(Wk, h0), hrows(Wk, h1),
            hrows(Wv, h0), hrows(Wv, h1), hrows(Wv, h2),
        ], axis=0)
        wt = np.ascontiguousarray(wt.T).astype(BF16NP)

        biasqk = np.stack([
            np.concatenate([hbias(bq, h0), hbias(bq, h1)]) * SCALE,
            np.concatenate([hbias(bq, h2) * SCALE, hbias(bk, h2)]),
            np.concatenate([hbias(bk, h0), hbias(bk, h1)]),
        ], axis=1).astype(np.float32)

        wo01 = np.concatenate([
            W_out[:, h0 * DH:(h0 + 1) * DH].T,
            W_out[:, h1 * DH:(h1 + 1) * DH].T,
        ], axis=0)
        wo01 = np.ascontiguousarray(wo01).astype(BF16NP)
        wo2 = np.ascontiguousarray(
            W_out[:, h2 * DH:(h2 + 1) * DH].T).astype(BF16NP)

        in_maps.append({
            "qT": qT, "wt": wt, "biasqk": biasqk,
            "wo01": wo01, "wo2": wo2,
        })
    return in_maps


_NC = None


def _get_nc():
    global _NC
    if _NC is None:
        _NC = build_program()
    return _NC


def kernel(q, k, v, W_qkv, b_qkv, W_out, b_out, _trace=False):
    nc = _get_nc()
    in_maps = make_core_inputs(q, W_qkv, b_qkv, W_out, b_out)
    res = bass_utils.run_bass_kernel_spmd(
        nc, in_maps, core_ids=list(range(NCORES)), trace=_trace)
    kernel.last_result = res
    W_out = np.asarray(W_out, np.float32)
    bv = np.asarray(b_qkv, np.float32)[2 * D:3 * D]
    bias = np.asarray(b_out, np.float32) + W_out @ bv
    y = np.empty((B, S, D), np.float32)
    for b in range(B):
        acc = res.results[4 * b]["yT"].astype(np.float32)
        for g in range(1, 4):
            acc = acc + res.results[4 * b + g]["yT"]
        y[b] = acc.T + bias
    return y
